# revision 31
# baseline (speedup 1.0000x reference)
"""Distributed Trainium2 kernel for the fused attention-autoencoder layer.

Reference math (per head h):
  Q = x @ Wq_h^T + bq_h ; K = x @ Wk_h^T + bk_h ; V = x @ Wv_h^T + bv_h
  scores = K^T Q / sqrt(E); A = softmax(scores, -1); Zh = V @ A
  O = concat_h(Zh) @ Wz^T + bz ; LN1 = ln(O)*g1+b1 + x
  FN = LN1 @ Wf^T + bf ; out = ln(FN)*g2+b2 + LN1

Restructuring (head h lives on core h):
  With xa = [x | 1] (augmented) and G~ = xa^T xa (symmetric):
    scores_h = Wka_h G~ Wqa_h^T / sqrt(E)  where Wka = [Wk|bk], Wqa = [Wq|bq]
  G~ rows are computed head-parallel over the FULL sequence (core h does
  rows [128h,128h+128) x all cols) and AllGathered; the ones-row (row E)
  is reconstructed locally from the gathered column E by symmetry.
  A_h = softmax(scores_h) (normalized in place). Then with
    D_h = A_h^T [Wv_h | bv_h]   (no transpose of A needed)
    C_h = D_h^T Wz_h^T,  r_h = D_h[:,E]^T Wz_h^T + bz/8
  we use  O = sum_h (V_h A_h Wz_h^T) = x (sum_h C_h) + 1 (sum_h r_h)^T,
  so the cross-core reduction is an AllReduce of the small [E+1, E]
  matrix [C_h; r_h] (done in two column-half chunks, overlapped with
  compute) instead of a ReduceScatter of the [S, E] output. Each core
  then computes ONLY its own SS=512-row shard: O_s = x_s Csum + 1 rsum^T
  and runs LN1/FFN/LN2 locally on those rows. Host concatenates shards.
"""

import numpy as np
import ml_dtypes

import concourse.bass as bass
import concourse.mybir as mybir
import concourse.tile as tile
from concourse import bacc
from concourse.bass_utils import run_bass_kernel_spmd
from concourse.masks import make_identity

S, E, H = 4096, 1024, 8
P = 128
EA = 1152          # augmented (E + ones col) padded to 9*128 (weights only)
GW = E + 8         # G col width: E cols + ones col at E, padded to 1032
NET = E // P       # 8
NAT = EA // P      # 9
NKT = S // P       # 32 sequence tiles (full S)
SS = S // H        # 512 rows per core (contiguous shard)
NSS = SS // P      # 4
NH = 2             # 512-wide free-dim halves of E
WVW = E + 8        # wva width: Wv cols + bv col at E, padded to 1032
EPS = 1e-5
SCALE = 1.0 / 32.0  # 1/sqrt(E)

F32 = mybir.dt.float32
BF16 = mybir.dt.bfloat16

# packed rows input: [bz/8, g1, b1, bf, g2, b2]; rows_bc holds the last 5
L_G1, L_B1, L_BF, L_G2, L_B2 = range(5)

LAST_RESULT = None  # test harness reads exec_time_ns off this


def _bcast_row(t: bass.AP) -> bass.AP:
    """[1, n] DRAM row -> partition-broadcast AP."""
    return bass.AP(tensor=t.tensor, offset=t.offset, ap=[[0, P], [1, t.shape[-1]]])


def build_nc(id_g1b1=False, id_g2b2=False):
    nc = bacc.Bacc(num_devices=H)

    # pre-tiled partition-major layouts: row p holds xa[k*P+p, :] for all k
    xap = nc.declare_dram_parameter("xap", [P, NKT * GW], BF16, isOutput=False)
    xcp = nc.declare_dram_parameter("xcp", [P, NKT * P], BF16, isOutput=False)
    wqa = nc.declare_dram_parameter("wqa", [EA, E], BF16, isOutput=False)
    wka = nc.declare_dram_parameter("wka", [EA, E], BF16, isOutput=False)
    wva = nc.declare_dram_parameter("wva", [E, WVW], BF16, isOutput=False)
    wzT = nc.declare_dram_parameter("wzT", [E, E], BF16, isOutput=False)
    wfT = nc.declare_dram_parameter("wfT", [E, E], BF16, isOutput=False)
    xts = nc.declare_dram_parameter("xts", [E, SS], BF16, isOutput=False)
    xs = nc.declare_dram_parameter("xs", [SS, E], F32, isOutput=False)
    rows = nc.declare_dram_parameter("rows", [6, E], F32, isOutput=False)
    out = nc.declare_dram_parameter("out", [SS, E], F32, isOutput=True)

    g_part = nc.dram_tensor("g_part", [P, GW], BF16)
    g_full = nc.dram_tensor("g_full", [E, GW], BF16, addr_space="Shared")
    # C reduction as explicit RS+AG (faster than fused AR on this stack);
    # rows padded to 1032 so the 8-way partition split is exact.
    CR = E + 8
    c_part = [nc.dram_tensor(f"c_part{n}", [CR, 512], BF16) for n in range(NH)]
    c_full = [
        nc.dram_tensor(f"c_full{n}", [CR, 512], BF16, addr_space="Shared")
        for n in range(NH)
    ]

    rg = [list(range(H))]

    with tile.TileContext(nc) as tc:
        with (
            tc.tile_pool(name="singles", bufs=1) as singles,
            tc.tile_pool(name="stat", bufs=4) as stat,
            tc.tile_pool(name="ps_mm", bufs=6, space="PSUM") as ps_mm,
            tc.tile_pool(name="ps_tr", bufs=2, space="PSUM") as ps_tr,
        ):
            ident = singles.tile([P, P], BF16)
            identf = singles.tile([P, P], F32)
            bz8_sb = singles.tile([1, E], F32)
            eps_sb = singles.tile([P, 1], F32)

            with tc.tile_pool(name="pc", bufs=1) as pc:
                c_sb = pc.tile([P, NET, 512], BF16)
                xts_sb = pc.tile([P, NET, SS], BF16)
                z7_sb = pc.tile([CR - E - 1, 512], BF16)
                with tc.tile_pool(name="pwz", bufs=1) as pwz:
                    wva_sb = pwz.tile([P, NET, WVW], BF16)
                    wzT_sb = pwz.tile([P, NET, E], BF16)
                    with tc.tile_pool(name="pd", bufs=1) as pd:
                        d_sb = pd.tile([P, NET, WVW], BF16)
                        with tc.tile_pool(name="pa", bufs=1) as pa:
                            a_sb = pa.tile([P, NET, E], BF16)
                            with tc.tile_pool(name="pwqk", bufs=1) as pwqk:
                                wqa_sb = pwqk.tile([P, NAT, E], BF16)
                                wka_sb = pwqk.tile([P, NAT, E], BF16)
                                u_sb = pwqk.tile([P, NAT, E], BF16)
                                gx8_sb = pwqk.tile([P, GW], BF16)
                                with tc.tile_pool(name="pg", bufs=1) as pg:
                                    g_sb = pg.tile([P, NET, GW], BF16)

                                    # ===== phase 1: G row-block over full S =====
                                    KCH = 2  # k-tiles per streamed chunk
                                    with tc.tile_pool(name="pxa", bufs=1) as pxa, \
                                         tc.tile_pool(name="pxs", bufs=6) as pxs:
                                        xcol_sb = pxa.tile([P, NKT, P], BF16)
                                        nc.sync.dma_start(
                                            out=xcol_sb,
                                            in_=xcp[:, :].rearrange(
                                                "p (t c) -> p t c", c=P
                                            ),
                                        )
                                        gchunks = [(0, 512), (512, 512), (1024, GW - E)]
                                        psg = [
                                            ps_mm.tile([P, w], F32, tag="mm",
                                                       name=f"psg_{i}")
                                            for i, (o, w) in enumerate(gchunks)
                                        ]
                                        for c in range(NKT // KCH):
                                            xa_t = pxs.tile(
                                                [P, KCH, GW], BF16, tag="xat"
                                            )
                                            nc.sync.dma_start(
                                                out=xa_t,
                                                in_=xap[
                                                    :, c * KCH * GW : (c + 1) * KCH * GW
                                                ].rearrange("p (t e) -> p t e", e=GW),
                                            )
                                            for kk in range(KCH):
                                                k = c * KCH + kk
                                                for i, (o, w) in enumerate(gchunks):
                                                    nc.tensor.matmul(
                                                        psg[i],
                                                        xcol_sb[:, k, :],
                                                        xa_t[:, kk, o : o + w],
                                                        start=(k == 0),
                                                        stop=(k == NKT - 1),
                                                    )
                                        gp = pxa.tile([P, GW], BF16)
                                        for i, (o, w) in enumerate(gchunks):
                                            nc.vector.tensor_copy(
                                                out=gp[:, o : o + w], in_=psg[i]
                                            )
                                        nc.sync.dma_start(out=g_part[:, :], in_=gp)
                                        nc.gpsimd.collective_compute(
                                            "AllGather",
                                            mybir.AluOpType.bypass,
                                            replica_groups=rg,
                                            ins=[g_part[:, :]],
                                            outs=[g_full[:, :]],
                                        )

                                        # constants + phase-2/3 weights (after
                                        # the collective: G path wins DMA prio)
                                        make_identity(nc, ident)
                                        make_identity(nc, identf)
                                        nc.sync.dma_start(
                                            out=bz8_sb, in_=rows[0:1, :]
                                        )
                                        nc.vector.memset(eps_sb, EPS)
                                        nc.sync.dma_start(
                                            out=wqa_sb,
                                            in_=wqa[:, :].rearrange(
                                                "(t p) e -> p t e", p=P
                                            ),
                                        )
                                        nc.sync.dma_start(
                                            out=wka_sb,
                                            in_=wka[:, :].rearrange(
                                                "(t p) e -> p t e", p=P
                                            ),
                                        )
                                        nc.sync.dma_start(
                                            out=wva_sb,
                                            in_=wva[:, :].rearrange(
                                                "(t p) e -> p t e", p=P
                                            ),
                                        )
                                        nc.sync.dma_start(
                                            out=wzT_sb,
                                            in_=wzT[:, :].rearrange(
                                                "(t p) e -> p t e", p=P
                                            ),
                                        )
                                        # Opart's lhsT prefetched early so it
                                        # never collides with the AR-gated
                                        # csum loads later
                                        nc.sync.dma_start(
                                            out=xts_sb,
                                            in_=xts[:, :].rearrange(
                                                "(t p) s -> p t s", p=P
                                            ),
                                        )
                                        nc.vector.memset(z7_sb, 0.0)
                                        for n in range(NH):
                                            nc.sync.dma_start(
                                                out=c_part[n][E + 1 : CR, :],
                                                in_=z7_sb,
                                            )

                                    # ===== gather G; rebuild ones-row =====
                                    nc.sync.dma_start(
                                        out=g_sb,
                                        in_=g_full[:, :].rearrange(
                                            "(t p) e -> p t e", p=P
                                        ),
                                    )
                                    nc.vector.memset(gx8_sb, 0.0)
                                    for t in range(NET):
                                        pst = ps_tr.tile(
                                            [1, P], BF16, tag="tr", name="pst"
                                        )
                                        nc.tensor.transpose(
                                            pst, g_sb[:, t, E : E + 1], ident
                                        )
                                        nc.vector.tensor_copy(
                                            out=gx8_sb[0:1, t * P : (t + 1) * P],
                                            in_=pst,
                                        )
                                    nc.vector.memset(gx8_sb[0:1, E : E + 1], float(S))

                                    # ===== phase 2: U = G~ @ Wqa =====
                                    # m-tile col slices of G~ (m=8: 8-wide tail)
                                    def gcols(m):
                                        return (m * P, min((m + 1) * P, GW))

                                    nc.vector.memset(u_sb[:, NET, :], 0.0)
                                    for (m0, m1) in [(0, 3), (3, 6), (6, 9)]:
                                        pss = {}
                                        for m in range(m0, m1):
                                            for n in range(NH):
                                                pss[m, n] = ps_mm.tile(
                                                    [P, 512], F32, tag="mm",
                                                    name=f"psu_{m}_{n}",
                                                )
                                        for k in range(NAT):
                                            for m in range(m0, m1):
                                                c0, c1 = gcols(m)
                                                mw = c1 - c0
                                                lhs = (
                                                    g_sb[:, k, c0:c1]
                                                    if k < NET
                                                    else gx8_sb[:, c0:c1]
                                                )
                                                for n in range(NH):
                                                    nc.tensor.matmul(
                                                        pss[m, n][0:mw, :],
                                                        lhs,
                                                        wqa_sb[:, k, n * 512 : (n + 1) * 512],
                                                        start=(k == 0),
                                                        stop=(k == NAT - 1),
                                                    )
                                        for m in range(m0, m1):
                                            c0, c1 = gcols(m)
                                            mw = c1 - c0
                                            for n in range(NH):
                                                nc.vector.tensor_copy(
                                                    out=u_sb[0:mw, m, n * 512 : (n + 1) * 512],
                                                    in_=pss[m, n][0:mw, :],
                                                )

                                # ===== phase 3: scores + softmax (normalized A) =====
                                with tc.tile_pool(name="p3", bufs=3) as p3:
                                    for m in range(NET):
                                        pss = [
                                            ps_mm.tile([P, 512], F32, tag="mm",
                                                       name=f"pssc_{n}")
                                            for n in range(NH)
                                        ]
                                        for k in range(NAT):
                                            lhs = wka_sb[:, k, m * P : (m + 1) * P]
                                            for n in range(NH):
                                                nc.tensor.matmul(
                                                    pss[n], lhs,
                                                    u_sb[:, k, n * 512 : (n + 1) * 512],
                                                    start=(k == 0), stop=(k == NAT - 1),
                                                )
                                        mxs = stat.tile([P, NH], F32, tag="mxs")
                                        for n in range(NH):
                                            nc.vector.reduce_max(
                                                out=mxs[:, n : n + 1], in_=pss[n],
                                                axis=mybir.AxisListType.X,
                                            )
                                        mx = stat.tile([P, 1], F32, tag="mx")
                                        nc.vector.tensor_max(
                                            mx, mxs[:, 0:1], mxs[:, 1:2]
                                        )
                                        negmx = stat.tile([P, 1], F32, tag="negmx")
                                        nc.vector.tensor_scalar_mul(negmx, mx, -SCALE)
                                        a_tmp = p3.tile([P, E], BF16, tag="atmp")
                                        rsums = stat.tile([P, NH], F32, tag="rsums")
                                        for n in range(NH):
                                            nc.scalar.activation(
                                                out=a_tmp[:, n * 512 : (n + 1) * 512],
                                                in_=pss[n],
                                                func=mybir.ActivationFunctionType.Exp,
                                                bias=negmx, scale=SCALE,
                                                accum_out=rsums[:, n : n + 1],
                                            )
                                        rsum = stat.tile([P, 1], F32, tag="rsum")
                                        nc.vector.tensor_add(
                                            rsum, rsums[:, 0:1], rsums[:, 1:2]
                                        )
                                        rcp = stat.tile([P, 1], F32, tag="rcp")
                                        nc.vector.reciprocal(out=rcp, in_=rsum)
                                        nc.vector.tensor_scalar_mul(
                                            a_sb[:, m, :], a_tmp, rcp
                                        )

                            # ===== phase 4a: D = A^T @ [Wv|bv] =====
                            dchunks = [(0, 512), (512, 512), (1024, WVW - E)]
                            for m in range(NET):
                                psd = [
                                    ps_mm.tile([P, w], F32, tag="mm",
                                               name=f"psd_{i}")
                                    for i, (o, w) in enumerate(dchunks)
                                ]
                                for k in range(NET):
                                    lhs = a_sb[:, k, m * P : (m + 1) * P]
                                    for i, (o, w) in enumerate(dchunks):
                                        nc.tensor.matmul(
                                            psd[i], lhs, wva_sb[:, k, o : o + w],
                                            start=(k == 0), stop=(k == NET - 1),
                                        )
                                for i, (o, w) in enumerate(dchunks):
                                    nc.vector.tensor_copy(
                                        out=d_sb[:, m, o : o + w], in_=psd[i]
                                    )

                        # ===== phase 4b: C = D^T @ WzT + r row; chunked AR =====
                        for n in range(NH):
                            for m in range(NET):
                                ps = ps_mm.tile([P, 512], F32, tag="mm",
                                                name=f"psc_{m}")
                                for k in range(NET):
                                    nc.tensor.matmul(
                                        ps,
                                        d_sb[:, k, m * P : (m + 1) * P],
                                        wzT_sb[:, k, n * 512 : (n + 1) * 512],
                                        start=(k == 0), stop=(k == NET - 1),
                                    )
                                nc.vector.tensor_copy(out=c_sb[:, m, :], in_=ps)
                            psr = ps_mm.tile([1, 512], F32, tag="mm", name="psr")
                            for k in range(NET):
                                nc.tensor.matmul(
                                    psr,
                                    d_sb[:, k, E : E + 1],
                                    wzT_sb[:, k, n * 512 : (n + 1) * 512],
                                    start=(k == 0), stop=(k == NET - 1),
                                )
                            rrow = stat.tile([1, 512], BF16, tag="rrow")
                            nc.vector.tensor_add(
                                rrow, psr, bz8_sb[:, n * 512 : (n + 1) * 512]
                            )
                            nc.sync.dma_start(
                                out=c_part[n][0:E, :].rearrange(
                                    "(t p) c -> p t c", p=P
                                ),
                                in_=c_sb,
                            )
                            nc.sync.dma_start(
                                out=c_part[n][E : E + 1, :], in_=rrow
                            )
                            nc.gpsimd.collective_compute(
                                "AllReduce",
                                mybir.AluOpType.add,
                                replica_groups=rg,
                                ins=[c_part[n][:, :]],
                                outs=[c_full[n][:, :]],
                            )

                # ===== phase 5: shard O = xs@Csum + r; LN1/FFN/LN2 =====
                with tc.tile_pool(name="p5", bufs=1) as p5, \
                     tc.tile_pool(name="p7", bufs=4) as p7:
                    wfT_sb = p5.tile([P, NET, E], BF16)
                    rows_bc = p5.tile([P, 5, E], F32)
                    xs_sb = p5.tile([P, NSS, E], F32)
                    csum_sb = [
                        p5.tile([P, NET, 512], BF16, name=f"csum{n}")
                        for n in range(NH)
                    ]
                    rbc_sb = [
                        p5.tile([P, 512], F32, name=f"rbc{n}") for n in range(NH)
                    ]
                    nc.sync.dma_start(
                        out=wfT_sb, in_=wfT[:, :].rearrange("(t p) e -> p t e", p=P)
                    )
                    for k in range(5):
                        nc.sync.dma_start(
                            out=rows_bc[:, k, :], in_=_bcast_row(rows[k + 1 : k + 2, :])
                        )
                    nc.sync.dma_start(
                        out=xs_sb, in_=xs[:, :].rearrange("(t p) e -> p t e", p=P)
                    )
                    o_sb = p5.tile([P, NSS, E], F32)
                    ln1_sb = p5.tile([P, NSS, E], F32)
                    l1t_sb = p5.tile([P, NET, SS], BF16)

                    bsts = [
                        stat.tile([P, 2, 6], F32, tag="bst", name=f"bst1_{st}")
                        for st in range(NSS)
                    ]
                    for n in range(NH):
                        rb_bf = stat.tile([P, 512], BF16, tag="rbbf")
                        nc.sync.dma_start(
                            out=rb_bf, in_=_bcast_row(c_full[n][E : E + 1, :])
                        )
                        for kh in range(2):
                            nc.sync.dma_start(
                                out=csum_sb[n][:, kh * 4 : (kh + 1) * 4, :],
                                in_=c_full[n][kh * 512 : (kh + 1) * 512, :].rearrange(
                                    "(t p) c -> p t c", p=P
                                ),
                            )
                        nc.vector.tensor_copy(out=rbc_sb[n], in_=rb_bf)
                        for m in range(NSS):
                            ps = ps_mm.tile([P, 512], F32, tag="mm",
                                            name=f"pso_{m}")
                            for k in range(NET):
                                nc.tensor.matmul(
                                    ps,
                                    xts_sb[:, k, m * P : (m + 1) * P],
                                    csum_sb[n][:, k, :],
                                    start=(k == 0), stop=(k == NET - 1),
                                )
                            nc.vector.tensor_add(
                                o_sb[:, m, n * 512 : (n + 1) * 512], ps, rbc_sb[n]
                            )
                        if n == 0:
                            # stats of the left half run under the 2nd AR wait
                            for st in range(NSS):
                                nc.vector.bn_stats(
                                    out=bsts[st][:, 0, :], in_=o_sb[:, st, 0:512]
                                )

                    def ln_core(dst, src, bst, r_g, r_b, skip_gb):
                        mv = stat.tile([P, 2], F32, tag="mv")
                        nc.vector.bn_aggr(out=mv, in_=bst)
                        sd = stat.tile([P, 1], F32, tag="sd")
                        nc.scalar.activation(
                            out=sd, in_=mv[:, 1:2],
                            func=mybir.ActivationFunctionType.Sqrt, bias=eps_sb[:, :],
                        )
                        rstd = stat.tile([P, 1], F32, tag="rstd")
                        nc.vector.reciprocal(out=rstd, in_=sd)
                        nc.vector.tensor_scalar(
                            out=dst, in0=src, scalar1=mv[:, 0:1], scalar2=rstd,
                            op0=mybir.AluOpType.subtract, op1=mybir.AluOpType.mult,
                        )
                        if not skip_gb:
                            nc.vector.tensor_mul(dst, dst, rows_bc[:, r_g, :])
                            nc.vector.tensor_add(dst, dst, rows_bc[:, r_b, :])

                    # stage-major tail: long independent runs per engine
                    for st in range(NSS):
                        nc.vector.bn_stats(
                            out=bsts[st][:, 1, :], in_=o_sb[:, st, 512:E]
                        )
                        t1 = ln1_sb[:, st, :]
                        ln_core(t1, o_sb[:, st, :], bsts[st], L_G1, L_B1, id_g1b1)
                        nc.vector.tensor_add(t1, t1, xs_sb[:, st, :])
                    for st in range(NSS):
                        for eb in range(NET):
                            pstf = ps_tr.tile([P, P], F32, tag="tr", name="pstf")
                            nc.tensor.transpose(
                                pstf, ln1_sb[:, st, eb * P : (eb + 1) * P], identf
                            )
                            nc.vector.tensor_copy(
                                out=l1t_sb[:, eb, st * P : (st + 1) * P], in_=pstf
                            )
                    # FFN -> LN2 -> out interleaved per tile so LN2(st) never
                    # queues behind later tiles' FFN completions
                    for st in range(NSS):
                        f1 = p7.tile([P, E], F32, tag="f1", name=f"f1_{st}")
                        bst2 = stat.tile([P, 2, 6], F32, tag="bst2",
                                         name=f"bst2_{st}")
                        for n in range(NH):
                            ps = ps_mm.tile([P, 512], F32, tag="mm",
                                            name=f"psf_{n}")
                            for k in range(NET):
                                nc.tensor.matmul(
                                    ps,
                                    l1t_sb[:, k, st * P : (st + 1) * P],
                                    wfT_sb[:, k, n * 512 : (n + 1) * 512],
                                    start=(k == 0), stop=(k == NET - 1),
                                )
                            nc.vector.tensor_add(
                                f1[:, n * 512 : (n + 1) * 512],
                                ps,
                                rows_bc[:, L_BF, n * 512 : (n + 1) * 512],
                            )
                            nc.vector.bn_stats(
                                out=bst2[:, n, :],
                                in_=f1[:, n * 512 : (n + 1) * 512],
                            )
                        fo = p7.tile([P, E], F32, tag="fo", name=f"fo_{st}")
                        ln_core(fo, f1, bst2, L_G2, L_B2, id_g2b2)
                        nc.vector.tensor_add(fo, fo, ln1_sb[:, st, :])
                        nc.sync.dma_start(out=out[st * P : (st + 1) * P, :], in_=fo)

    nc.finalize()
    return nc


_NC_CACHE = None


def kernel(**inputs) -> np.ndarray:
    global _NC_CACHE, LAST_RESULT
    x = np.asarray(inputs["x"], np.float32)
    Wq = np.asarray(inputs["Wq"], np.float32)
    bq = np.asarray(inputs["bq"], np.float32)
    Wk = np.asarray(inputs["Wk"], np.float32)
    bk = np.asarray(inputs["bk"], np.float32)
    Wv = np.asarray(inputs["Wv"], np.float32)
    bv = np.asarray(inputs["bv"], np.float32)
    Wz = np.asarray(inputs["Wz"], np.float32)
    bz = np.asarray(inputs["bz"], np.float32)
    g1 = np.asarray(inputs["g1"], np.float32)
    b1 = np.asarray(inputs["b1"], np.float32)
    Wf = np.asarray(inputs["Wf"], np.float32)
    bf_ = np.asarray(inputs["bf"], np.float32)
    g2 = np.asarray(inputs["g2"], np.float32)
    b2 = np.asarray(inputs["b2"], np.float32)

    BF = ml_dtypes.bfloat16
    id_g1b1 = bool(np.all(g1 == 1.0) and np.all(b1 == 0.0))
    id_g2b2 = bool(np.all(g2 == 1.0) and np.all(b2 == 0.0))
    key = (id_g1b1, id_g2b2)
    if _NC_CACHE is None or _NC_CACHE[0] != key:
        _NC_CACHE = (key, build_nc(id_g1b1, id_g2b2))
    nc = _NC_CACHE[1]

    xa_np = np.concatenate(
        [x, np.ones((S, 1), np.float32), np.zeros((S, GW - E - 1), np.float32)],
        axis=1,
    ).astype(BF)
    # partition-major pre-tiling: row p <- xa[k*P+p, :] for k = 0..NKT-1
    xap_np = np.ascontiguousarray(
        xa_np.reshape(NKT, P, GW).transpose(1, 0, 2).reshape(P, NKT * GW)
    )
    xt_np = np.ascontiguousarray(x.T)
    wfT_np = np.ascontiguousarray(Wf.T).astype(BF)
    rows_np = np.ascontiguousarray(
        np.stack([bz / H, g1, b1, bf_, g2, b2]).astype(np.float32)
    )
    pad_w = np.zeros((EA - E - 1, E), np.float32)

    in_maps = []
    for h in range(H):
        wqa_h = np.concatenate([Wq[h].T, bq[h][None, :], pad_w], axis=0).astype(BF)
        wka_h = np.concatenate([Wk[h].T, bk[h][None, :], pad_w], axis=0).astype(BF)
        wva_h = np.concatenate(
            [Wv[h], bv[h][:, None], np.zeros((E, WVW - E - 1), np.float32)], axis=1
        ).astype(BF)
        wzT_h = np.ascontiguousarray(Wz[:, h * E : (h + 1) * E].T).astype(BF)
        xcp_h = np.ascontiguousarray(
            xa_np[:, h * P : (h + 1) * P]
            .reshape(NKT, P, P)
            .transpose(1, 0, 2)
            .reshape(P, NKT * P)
        )
        in_maps.append(
            {
                "xap": xap_np,
                "xcp": xcp_h,
                "wqa": np.ascontiguousarray(wqa_h),
                "wka": np.ascontiguousarray(wka_h),
                "wva": np.ascontiguousarray(wva_h),
                "wzT": wzT_h,
                "wfT": wfT_np,
                "xts": np.ascontiguousarray(xt_np[:, h * SS : (h + 1) * SS]).astype(BF),
                "xs": np.ascontiguousarray(x[h * SS : (h + 1) * SS]),
                "rows": rows_np,
            }
        )

    res = run_bass_kernel_spmd(nc, in_maps, list(range(H)))
    LAST_RESULT = res
    out = np.empty((S, E), np.float32)
    for h in range(H):
        out[h * SS : (h + 1) * SS] = res.results[h]["out"]
    return out


# revision 35
# speedup vs baseline: 1.0253x; 1.0253x over previous
"""Distributed Trainium2 kernel for the fused attention-autoencoder layer.

Reference math (per head h):
  Q = x @ Wq_h^T + bq_h ; K = x @ Wk_h^T + bk_h ; V = x @ Wv_h^T + bv_h
  scores = K^T Q / sqrt(E); A = softmax(scores, -1); Zh = V @ A
  O = concat_h(Zh) @ Wz^T + bz ; LN1 = ln(O)*g1+b1 + x
  FN = LN1 @ Wf^T + bf ; out = ln(FN)*g2+b2 + LN1

Restructuring (head h lives on core h):
  With xa = [x | 1] (augmented) and G~ = xa^T xa (symmetric):
    scores_h = Wka_h G~ Wqa_h^T / sqrt(E)  where Wka = [Wk|bk], Wqa = [Wq|bq]
  G~ rows are computed head-parallel over the FULL sequence (core h does
  rows [128h,128h+128) x all cols) and AllGathered; the ones-row (row E)
  is reconstructed locally from the gathered column E by symmetry.
  A_h = softmax(scores_h) (normalized in place). Then with
    D_h = A_h^T [Wv_h | bv_h]   (no transpose of A needed)
    C_h = D_h^T Wz_h^T,  r_h = D_h[:,E]^T Wz_h^T + bz/8
  we use  O = sum_h (V_h A_h Wz_h^T) = x (sum_h C_h) + 1 (sum_h r_h)^T,
  so the cross-core reduction is an AllReduce of the small [E+1, E]
  matrix [C_h; r_h] (done in two column-half chunks, overlapped with
  compute) instead of a ReduceScatter of the [S, E] output. Each core
  then computes ONLY its own SS=512-row shard: O_s = x_s Csum + 1 rsum^T
  and runs LN1/FFN/LN2 locally on those rows. Host concatenates shards.
"""

import numpy as np
import ml_dtypes

import concourse.bass as bass
import concourse.mybir as mybir
import concourse.tile as tile
from concourse import bacc
from concourse.bass_utils import run_bass_kernel_spmd
from concourse.masks import make_identity

S, E, H = 4096, 1024, 8
P = 128
EA = 1152          # augmented (E + ones col) padded to 9*128 (weights only)
GW = E + 8         # G col width: E cols + ones col at E, padded to 1032
NET = E // P       # 8
NAT = EA // P      # 9
NKT = S // P       # 32 sequence tiles (full S)
SS = S // H        # 512 rows per core (contiguous shard)
NSS = SS // P      # 4
NH = 2             # 512-wide free-dim halves of E
WVW = E + 8        # wva width: Wv cols + bv col at E, padded to 1032
EPS = 1e-5
SCALE = 1.0 / 32.0  # 1/sqrt(E)

F32 = mybir.dt.float32
BF16 = mybir.dt.bfloat16

# packed rows input: [bz/8, g1, b1, bf, g2, b2]; rows_bc holds the last 5
L_G1, L_B1, L_BF, L_G2, L_B2 = range(5)

LAST_RESULT = None  # test harness reads exec_time_ns off this


def _bcast_row(t: bass.AP) -> bass.AP:
    """[1, n] DRAM row -> partition-broadcast AP."""
    return bass.AP(tensor=t.tensor, offset=t.offset, ap=[[0, P], [1, t.shape[-1]]])


def build_nc(id_g1b1=False, id_g2b2=False):
    nc = bacc.Bacc(num_devices=H)

    # pre-tiled partition-major layouts: row p holds xa[k*P+p, :] for all k
    xap = nc.declare_dram_parameter("xap", [P, NKT * GW], BF16, isOutput=False)
    xcp = nc.declare_dram_parameter("xcp", [P, NKT * P], BF16, isOutput=False)
    wqa = nc.declare_dram_parameter("wqa", [EA, E], BF16, isOutput=False)
    wka = nc.declare_dram_parameter("wka", [EA, E], BF16, isOutput=False)
    wva = nc.declare_dram_parameter("wva", [E, WVW], BF16, isOutput=False)
    wzT = nc.declare_dram_parameter("wzT", [E, E], BF16, isOutput=False)
    wfT = nc.declare_dram_parameter("wfT", [E, E], BF16, isOutput=False)
    xts = nc.declare_dram_parameter("xts", [E, SS], BF16, isOutput=False)
    xs = nc.declare_dram_parameter("xs", [SS, E], BF16, isOutput=False)
    rows = nc.declare_dram_parameter("rows", [6, E], F32, isOutput=False)
    out = nc.declare_dram_parameter("out", [SS, E], F32, isOutput=True)

    g_part = nc.dram_tensor("g_part", [P, GW], BF16)
    g_full = nc.dram_tensor("g_full", [E, GW], BF16, addr_space="Shared")
    # C reduction as explicit RS+AG (faster than fused AR on this stack);
    # rows padded to 1032 so the 8-way partition split is exact.
    CR = E + 8
    c_part = [nc.dram_tensor(f"c_part{n}", [CR, 512], BF16) for n in range(NH)]
    c_full = [
        nc.dram_tensor(f"c_full{n}", [CR, 512], BF16, addr_space="Shared")
        for n in range(NH)
    ]

    rg = [list(range(H))]

    with tile.TileContext(nc) as tc:
        with (
            tc.tile_pool(name="singles", bufs=1) as singles,
            tc.tile_pool(name="stat", bufs=4) as stat,
            tc.tile_pool(name="ps_mm", bufs=6, space="PSUM") as ps_mm,
            tc.tile_pool(name="ps_tr", bufs=2, space="PSUM") as ps_tr,
        ):
            ident = singles.tile([P, P], BF16)
            identf = singles.tile([P, P], F32)
            bz8_sb = singles.tile([1, E], F32)
            eps_sb = singles.tile([P, 1], F32)

            with tc.tile_pool(name="pc", bufs=1) as pc:
                c_sb = pc.tile([P, NET, 512], BF16)
                xts_sb = pc.tile([P, NET, SS], BF16)
                z7_sb = pc.tile([CR - E - 1, 512], BF16)
                with tc.tile_pool(name="pwz", bufs=1) as pwz:
                    wva_sb = pwz.tile([P, NET, WVW], BF16)
                    wzT_sb = pwz.tile([P, NET, E], BF16)
                    with tc.tile_pool(name="pd", bufs=1) as pd:
                        d_sb = pd.tile([P, NET, WVW], BF16)
                        with tc.tile_pool(name="pa", bufs=1) as pa:
                            a_sb = pa.tile([P, NET, E], BF16)
                            with tc.tile_pool(name="pwqk", bufs=1) as pwqk:
                                wqa_sb = pwqk.tile([P, NAT, E], BF16)
                                wka_sb = pwqk.tile([P, NAT, E], BF16)
                                u_sb = pwqk.tile([P, NAT, E], BF16)
                                gx8_sb = pwqk.tile([P, GW], BF16)
                                with tc.tile_pool(name="pg", bufs=1) as pg:
                                    g_sb = pg.tile([P, NET, GW], BF16)

                                    # ===== phase 1: G row-block over full S =====
                                    KCH = 2  # k-tiles per streamed chunk
                                    with tc.tile_pool(name="pxa", bufs=1) as pxa, \
                                         tc.tile_pool(name="pxs", bufs=6) as pxs:
                                        xcol_sb = pxa.tile([P, NKT, P], BF16)
                                        nc.sync.dma_start(
                                            out=xcol_sb,
                                            in_=xcp[:, :].rearrange(
                                                "p (t c) -> p t c", c=P
                                            ),
                                        )
                                        gchunks = [(0, 512), (512, 512), (1024, GW - E)]
                                        psg = [
                                            ps_mm.tile([P, w], F32, tag="mm",
                                                       name=f"psg_{i}")
                                            for i, (o, w) in enumerate(gchunks)
                                        ]
                                        for c in range(NKT // KCH):
                                            xa_t = pxs.tile(
                                                [P, KCH, GW], BF16, tag="xat"
                                            )
                                            nc.sync.dma_start(
                                                out=xa_t,
                                                in_=xap[
                                                    :, c * KCH * GW : (c + 1) * KCH * GW
                                                ].rearrange("p (t e) -> p t e", e=GW),
                                            )
                                            for kk in range(KCH):
                                                k = c * KCH + kk
                                                for i, (o, w) in enumerate(gchunks):
                                                    nc.tensor.matmul(
                                                        psg[i],
                                                        xcol_sb[:, k, :],
                                                        xa_t[:, kk, o : o + w],
                                                        start=(k == 0),
                                                        stop=(k == NKT - 1),
                                                    )
                                        gp = pxa.tile([P, GW], BF16)
                                        for i, (o, w) in enumerate(gchunks):
                                            nc.vector.tensor_copy(
                                                out=gp[:, o : o + w], in_=psg[i]
                                            )
                                        nc.sync.dma_start(out=g_part[:, :], in_=gp)
                                        nc.gpsimd.collective_compute(
                                            "AllGather",
                                            mybir.AluOpType.bypass,
                                            replica_groups=rg,
                                            ins=[g_part[:, :]],
                                            outs=[g_full[:, :]],
                                        )

                                        # constants + phase-2/3 weights (after
                                        # the collective: G path wins DMA prio)
                                        make_identity(nc, ident)
                                        make_identity(nc, identf)
                                        nc.sync.dma_start(
                                            out=bz8_sb, in_=rows[0:1, :]
                                        )
                                        nc.vector.memset(eps_sb, EPS)
                                        nc.sync.dma_start(
                                            out=wqa_sb,
                                            in_=wqa[:, :].rearrange(
                                                "(t p) e -> p t e", p=P
                                            ),
                                        )
                                        nc.sync.dma_start(
                                            out=wka_sb,
                                            in_=wka[:, :].rearrange(
                                                "(t p) e -> p t e", p=P
                                            ),
                                        )
                                        nc.sync.dma_start(
                                            out=wva_sb,
                                            in_=wva[:, :].rearrange(
                                                "(t p) e -> p t e", p=P
                                            ),
                                        )
                                        nc.sync.dma_start(
                                            out=wzT_sb,
                                            in_=wzT[:, :].rearrange(
                                                "(t p) e -> p t e", p=P
                                            ),
                                        )
                                        # Opart's lhsT prefetched early so it
                                        # never collides with the AR-gated
                                        # csum loads later
                                        nc.sync.dma_start(
                                            out=xts_sb,
                                            in_=xts[:, :].rearrange(
                                                "(t p) s -> p t s", p=P
                                            ),
                                        )
                                        nc.vector.memset(z7_sb, 0.0)
                                        for n in range(NH):
                                            nc.sync.dma_start(
                                                out=c_part[n][E + 1 : CR, :],
                                                in_=z7_sb,
                                            )

                                    # ===== gather G; rebuild ones-row =====
                                    # split by columns: U's first m-group only
                                    # needs cols 0:512, so it starts sooner
                                    nc.sync.dma_start(
                                        out=g_sb[:, :, 0:512],
                                        in_=g_full[:, 0:512].rearrange(
                                            "(t p) e -> p t e", p=P
                                        ),
                                    )
                                    nc.sync.dma_start(
                                        out=g_sb[:, :, 512:GW],
                                        in_=g_full[:, 512:GW].rearrange(
                                            "(t p) e -> p t e", p=P
                                        ),
                                    )
                                    nc.vector.memset(gx8_sb, 0.0)
                                    for t in range(NET):
                                        pst = ps_tr.tile(
                                            [1, P], BF16, tag="tr", name="pst"
                                        )
                                        nc.tensor.transpose(
                                            pst, g_sb[:, t, E : E + 1], ident
                                        )
                                        nc.vector.tensor_copy(
                                            out=gx8_sb[0:1, t * P : (t + 1) * P],
                                            in_=pst,
                                        )
                                    nc.vector.memset(gx8_sb[0:1, E : E + 1], float(S))

                                    # ===== phase 2: U = G~ @ Wqa =====
                                    # m-tile col slices of G~ (m=8: 8-wide tail)
                                    def gcols(m):
                                        return (m * P, min((m + 1) * P, GW))

                                    nc.vector.memset(u_sb[:, NET, :], 0.0)
                                    for (m0, m1) in [(0, 3), (3, 6), (6, 9)]:
                                        pss = {}
                                        for m in range(m0, m1):
                                            for n in range(NH):
                                                pss[m, n] = ps_mm.tile(
                                                    [P, 512], F32, tag="mm",
                                                    name=f"psu_{m}_{n}",
                                                )
                                        for k in range(NAT):
                                            for m in range(m0, m1):
                                                c0, c1 = gcols(m)
                                                mw = c1 - c0
                                                lhs = (
                                                    g_sb[:, k, c0:c1]
                                                    if k < NET
                                                    else gx8_sb[:, c0:c1]
                                                )
                                                for n in range(NH):
                                                    nc.tensor.matmul(
                                                        pss[m, n][0:mw, :],
                                                        lhs,
                                                        wqa_sb[:, k, n * 512 : (n + 1) * 512],
                                                        start=(k == 0),
                                                        stop=(k == NAT - 1),
                                                    )
                                        for m in range(m0, m1):
                                            c0, c1 = gcols(m)
                                            mw = c1 - c0
                                            for n in range(NH):
                                                nc.vector.tensor_copy(
                                                    out=u_sb[0:mw, m, n * 512 : (n + 1) * 512],
                                                    in_=pss[m, n][0:mw, :],
                                                )

                                # ===== phase 3: scores + softmax (normalized A) =====
                                with tc.tile_pool(name="p3", bufs=3) as p3:
                                    for m in range(NET):
                                        pss = [
                                            ps_mm.tile([P, 512], F32, tag="mm",
                                                       name=f"pssc_{n}")
                                            for n in range(NH)
                                        ]
                                        for k in range(NAT):
                                            lhs = wka_sb[:, k, m * P : (m + 1) * P]
                                            for n in range(NH):
                                                nc.tensor.matmul(
                                                    pss[n], lhs,
                                                    u_sb[:, k, n * 512 : (n + 1) * 512],
                                                    start=(k == 0), stop=(k == NAT - 1),
                                                )
                                        mxs = stat.tile([P, NH], F32, tag="mxs")
                                        for n in range(NH):
                                            nc.vector.reduce_max(
                                                out=mxs[:, n : n + 1], in_=pss[n],
                                                axis=mybir.AxisListType.X,
                                            )
                                        mx = stat.tile([P, 1], F32, tag="mx")
                                        nc.vector.tensor_max(
                                            mx, mxs[:, 0:1], mxs[:, 1:2]
                                        )
                                        negmx = stat.tile([P, 1], F32, tag="negmx")
                                        nc.vector.tensor_scalar_mul(negmx, mx, -SCALE)
                                        a_tmp = p3.tile([P, E], BF16, tag="atmp")
                                        rsums = stat.tile([P, NH], F32, tag="rsums")
                                        for n in range(NH):
                                            nc.scalar.activation(
                                                out=a_tmp[:, n * 512 : (n + 1) * 512],
                                                in_=pss[n],
                                                func=mybir.ActivationFunctionType.Exp,
                                                bias=negmx, scale=SCALE,
                                                accum_out=rsums[:, n : n + 1],
                                            )
                                        rsum = stat.tile([P, 1], F32, tag="rsum")
                                        nc.vector.tensor_add(
                                            rsum, rsums[:, 0:1], rsums[:, 1:2]
                                        )
                                        rcp = stat.tile([P, 1], F32, tag="rcp")
                                        nc.vector.reciprocal(out=rcp, in_=rsum)
                                        nc.vector.tensor_scalar_mul(
                                            a_sb[:, m, :], a_tmp, rcp
                                        )

                            # ===== phase 4a: D = A^T @ [Wv|bv] =====
                            dchunks = [(0, 512), (512, 512), (1024, WVW - E)]
                            for m in range(NET):
                                psd = [
                                    ps_mm.tile([P, w], F32, tag="mm",
                                               name=f"psd_{i}")
                                    for i, (o, w) in enumerate(dchunks)
                                ]
                                for k in range(NET):
                                    lhs = a_sb[:, k, m * P : (m + 1) * P]
                                    for i, (o, w) in enumerate(dchunks):
                                        nc.tensor.matmul(
                                            psd[i], lhs, wva_sb[:, k, o : o + w],
                                            start=(k == 0), stop=(k == NET - 1),
                                        )
                                for i, (o, w) in enumerate(dchunks):
                                    nc.vector.tensor_copy(
                                        out=d_sb[:, m, o : o + w], in_=psd[i]
                                    )

                        # ===== phase 4b: C = D^T @ WzT + r row; chunked AR =====
                        for n in range(NH):
                            for m in range(NET):
                                ps = ps_mm.tile([P, 512], F32, tag="mm",
                                                name=f"psc_{m}")
                                for k in range(NET):
                                    nc.tensor.matmul(
                                        ps,
                                        d_sb[:, k, m * P : (m + 1) * P],
                                        wzT_sb[:, k, n * 512 : (n + 1) * 512],
                                        start=(k == 0), stop=(k == NET - 1),
                                    )
                                nc.vector.tensor_copy(out=c_sb[:, m, :], in_=ps)
                            psr = ps_mm.tile([1, 512], F32, tag="mm", name="psr")
                            for k in range(NET):
                                nc.tensor.matmul(
                                    psr,
                                    d_sb[:, k, E : E + 1],
                                    wzT_sb[:, k, n * 512 : (n + 1) * 512],
                                    start=(k == 0), stop=(k == NET - 1),
                                )
                            rrow = stat.tile([1, 512], BF16, tag="rrow")
                            nc.vector.tensor_add(
                                rrow, psr, bz8_sb[:, n * 512 : (n + 1) * 512]
                            )
                            nc.sync.dma_start(
                                out=c_part[n][0:E, :].rearrange(
                                    "(t p) c -> p t c", p=P
                                ),
                                in_=c_sb,
                            )
                            nc.sync.dma_start(
                                out=c_part[n][E : E + 1, :], in_=rrow
                            )
                            nc.gpsimd.collective_compute(
                                "AllReduce",
                                mybir.AluOpType.add,
                                replica_groups=rg,
                                ins=[c_part[n][:, :]],
                                outs=[c_full[n][:, :]],
                            )

                # ===== phase 5: shard O = xs@Csum + r; LN1/FFN/LN2 =====
                with tc.tile_pool(name="p5", bufs=1) as p5, \
                     tc.tile_pool(name="p7", bufs=4) as p7:
                    wfT_sb = p5.tile([P, NET, E], BF16)
                    rows_bc = p5.tile([P, 5, E], F32)
                    xs_sb = p5.tile([P, NSS, E], BF16)
                    csum_sb = [
                        p5.tile([P, NET, 512], BF16, name=f"csum{n}")
                        for n in range(NH)
                    ]
                    rbc_sb = [
                        p5.tile([P, 512], F32, name=f"rbc{n}") for n in range(NH)
                    ]
                    # xs+rows stream during AR0; wfT is emitted after csum0's
                    # AR0-gated wait, so its transfer runs in the AR1 window
                    # instead of stealing HBM bandwidth from AR0
                    nc.sync.dma_start(
                        out=xs_sb, in_=xs[:, :].rearrange("(t p) e -> p t e", p=P)
                    )
                    for k in range(5):
                        nc.sync.dma_start(
                            out=rows_bc[:, k, :], in_=_bcast_row(rows[k + 1 : k + 2, :])
                        )
                    o_sb = p5.tile([P, NSS, E], F32)
                    ln1_sb = p5.tile([P, NSS, E], F32)
                    l1t_sb = p5.tile([P, NET, SS], BF16)

                    bsts = [
                        stat.tile([P, 2, 6], F32, tag="bst", name=f"bst1_{st}")
                        for st in range(NSS)
                    ]
                    for n in range(NH):
                        rb_bf = stat.tile([P, 512], BF16, tag="rbbf")
                        nc.sync.dma_start(
                            out=rb_bf, in_=_bcast_row(c_full[n][E : E + 1, :])
                        )
                        for kh in range(2):
                            nc.sync.dma_start(
                                out=csum_sb[n][:, kh * 4 : (kh + 1) * 4, :],
                                in_=c_full[n][kh * 512 : (kh + 1) * 512, :].rearrange(
                                    "(t p) c -> p t c", p=P
                                ),
                            )
                        nc.vector.tensor_copy(out=rbc_sb[n], in_=rb_bf)
                        if n == 0:
                            nc.sync.dma_start(
                                out=wfT_sb,
                                in_=wfT[:, :].rearrange("(t p) e -> p t e", p=P),
                            )
                        for m in range(NSS):
                            ps = ps_mm.tile([P, 512], F32, tag="mm",
                                            name=f"pso_{m}")
                            for k in range(NET):
                                nc.tensor.matmul(
                                    ps,
                                    xts_sb[:, k, m * P : (m + 1) * P],
                                    csum_sb[n][:, k, :],
                                    start=(k == 0), stop=(k == NET - 1),
                                )
                            nc.vector.tensor_add(
                                o_sb[:, m, n * 512 : (n + 1) * 512], ps, rbc_sb[n]
                            )
                        if n == 0:
                            # stats of the left half run under the 2nd AR wait
                            for st in range(NSS):
                                nc.vector.bn_stats(
                                    out=bsts[st][:, 0, :], in_=o_sb[:, st, 0:512]
                                )

                    def ln_core(dst, src, bst, r_g, r_b, skip_gb):
                        mv = stat.tile([P, 2], F32, tag="mv")
                        nc.vector.bn_aggr(out=mv, in_=bst)
                        sd = stat.tile([P, 1], F32, tag="sd")
                        nc.scalar.activation(
                            out=sd, in_=mv[:, 1:2],
                            func=mybir.ActivationFunctionType.Sqrt, bias=eps_sb[:, :],
                        )
                        rstd = stat.tile([P, 1], F32, tag="rstd")
                        nc.vector.reciprocal(out=rstd, in_=sd)
                        nc.vector.tensor_scalar(
                            out=dst, in0=src, scalar1=mv[:, 0:1], scalar2=rstd,
                            op0=mybir.AluOpType.subtract, op1=mybir.AluOpType.mult,
                        )
                        if not skip_gb:
                            nc.vector.tensor_mul(dst, dst, rows_bc[:, r_g, :])
                            nc.vector.tensor_add(dst, dst, rows_bc[:, r_b, :])

                    # stage-major tail: long independent runs per engine
                    for st in range(NSS):
                        nc.vector.bn_stats(
                            out=bsts[st][:, 1, :], in_=o_sb[:, st, 512:E]
                        )
                        t1 = ln1_sb[:, st, :]
                        ln_core(t1, o_sb[:, st, :], bsts[st], L_G1, L_B1, id_g1b1)
                        nc.vector.tensor_add(t1, t1, xs_sb[:, st, :])
                    for st in range(NSS):
                        for eb in range(NET):
                            pstf = ps_tr.tile([P, P], F32, tag="tr", name="pstf")
                            nc.tensor.transpose(
                                pstf, ln1_sb[:, st, eb * P : (eb + 1) * P], identf
                            )
                            nc.vector.tensor_copy(
                                out=l1t_sb[:, eb, st * P : (st + 1) * P], in_=pstf
                            )
                    # FFN -> LN2 -> out interleaved per tile so LN2(st) never
                    # queues behind later tiles' FFN completions
                    for st in range(NSS):
                        f1 = p7.tile([P, E], F32, tag="f1", name=f"f1_{st}")
                        bst2 = stat.tile([P, 2, 6], F32, tag="bst2",
                                         name=f"bst2_{st}")
                        for n in range(NH):
                            ps = ps_mm.tile([P, 512], F32, tag="mm",
                                            name=f"psf_{n}")
                            for k in range(NET):
                                nc.tensor.matmul(
                                    ps,
                                    l1t_sb[:, k, st * P : (st + 1) * P],
                                    wfT_sb[:, k, n * 512 : (n + 1) * 512],
                                    start=(k == 0), stop=(k == NET - 1),
                                )
                            nc.vector.tensor_add(
                                f1[:, n * 512 : (n + 1) * 512],
                                ps,
                                rows_bc[:, L_BF, n * 512 : (n + 1) * 512],
                            )
                            nc.vector.bn_stats(
                                out=bst2[:, n, :],
                                in_=f1[:, n * 512 : (n + 1) * 512],
                            )
                        fo = p7.tile([P, E], F32, tag="fo", name=f"fo_{st}")
                        ln_core(fo, f1, bst2, L_G2, L_B2, id_g2b2)
                        nc.vector.tensor_add(fo, fo, ln1_sb[:, st, :])
                        nc.sync.dma_start(out=out[st * P : (st + 1) * P, :], in_=fo)

    nc.finalize()
    return nc


_NC_CACHE = None


def kernel(**inputs) -> np.ndarray:
    global _NC_CACHE, LAST_RESULT
    x = np.asarray(inputs["x"], np.float32)
    Wq = np.asarray(inputs["Wq"], np.float32)
    bq = np.asarray(inputs["bq"], np.float32)
    Wk = np.asarray(inputs["Wk"], np.float32)
    bk = np.asarray(inputs["bk"], np.float32)
    Wv = np.asarray(inputs["Wv"], np.float32)
    bv = np.asarray(inputs["bv"], np.float32)
    Wz = np.asarray(inputs["Wz"], np.float32)
    bz = np.asarray(inputs["bz"], np.float32)
    g1 = np.asarray(inputs["g1"], np.float32)
    b1 = np.asarray(inputs["b1"], np.float32)
    Wf = np.asarray(inputs["Wf"], np.float32)
    bf_ = np.asarray(inputs["bf"], np.float32)
    g2 = np.asarray(inputs["g2"], np.float32)
    b2 = np.asarray(inputs["b2"], np.float32)

    BF = ml_dtypes.bfloat16
    id_g1b1 = bool(np.all(g1 == 1.0) and np.all(b1 == 0.0))
    id_g2b2 = bool(np.all(g2 == 1.0) and np.all(b2 == 0.0))
    key = (id_g1b1, id_g2b2)
    if _NC_CACHE is None or _NC_CACHE[0] != key:
        _NC_CACHE = (key, build_nc(id_g1b1, id_g2b2))
    nc = _NC_CACHE[1]

    xa_np = np.concatenate(
        [x, np.ones((S, 1), np.float32), np.zeros((S, GW - E - 1), np.float32)],
        axis=1,
    ).astype(BF)
    # partition-major pre-tiling: row p <- xa[k*P+p, :] for k = 0..NKT-1
    xap_np = np.ascontiguousarray(
        xa_np.reshape(NKT, P, GW).transpose(1, 0, 2).reshape(P, NKT * GW)
    )
    xt_np = np.ascontiguousarray(x.T)
    wfT_np = np.ascontiguousarray(Wf.T).astype(BF)
    rows_np = np.ascontiguousarray(
        np.stack([bz / H, g1, b1, bf_, g2, b2]).astype(np.float32)
    )
    pad_w = np.zeros((EA - E - 1, E), np.float32)

    in_maps = []
    for h in range(H):
        wqa_h = np.concatenate([Wq[h].T, bq[h][None, :], pad_w], axis=0).astype(BF)
        wka_h = np.concatenate([Wk[h].T, bk[h][None, :], pad_w], axis=0).astype(BF)
        wva_h = np.concatenate(
            [Wv[h], bv[h][:, None], np.zeros((E, WVW - E - 1), np.float32)], axis=1
        ).astype(BF)
        wzT_h = np.ascontiguousarray(Wz[:, h * E : (h + 1) * E].T).astype(BF)
        xcp_h = np.ascontiguousarray(
            xa_np[:, h * P : (h + 1) * P]
            .reshape(NKT, P, P)
            .transpose(1, 0, 2)
            .reshape(P, NKT * P)
        )
        in_maps.append(
            {
                "xap": xap_np,
                "xcp": xcp_h,
                "wqa": np.ascontiguousarray(wqa_h),
                "wka": np.ascontiguousarray(wka_h),
                "wva": np.ascontiguousarray(wva_h),
                "wzT": wzT_h,
                "wfT": wfT_np,
                "xts": np.ascontiguousarray(xt_np[:, h * SS : (h + 1) * SS]).astype(BF),
                "xs": np.ascontiguousarray(x[h * SS : (h + 1) * SS]).astype(BF),
                "rows": rows_np,
            }
        )

    res = run_bass_kernel_spmd(nc, in_maps, list(range(H)))
    LAST_RESULT = res
    out = np.empty((S, E), np.float32)
    for h in range(H):
        out[h * SS : (h + 1) * SS] = res.results[h]["out"]
    return out
